# revision 8
# baseline (speedup 1.0000x reference)
"""Trainium2 Bass kernel for CoreSageLayer (GNN mean-aggregate + 3-way linear).

Computation (reference):
    mask = (adj == 1)                      # [N, N] 0/1
    deg  = mask.sum(axis=1)                # [N]
    x1   = (mask @ x) / deg[:, None]       # [N, F]
    out[k] = concat([x1, x], 1) @ W[k] + bias   # [3, N, O]

Distribution: row-shard adj / x1 / out over nodes across 8 cores; replicate
x and weights; no collectives (rows independent).

Device schedule per core (nodes NB=1024, 8 node-tiles of 128):
  stage 1 (per node-tile j): fp8e4m3 DoubleRow matmuls — each instruction
      contracts a 256-neighbor chunk-pair (2 fp8 rows per PE cell, 0.5
      cycles per output column). lhsT = maskT chunk [128, 2, 128], rhs =
      [1 | x] fp8 chunk. Column 0 of the moving tensor is the ones column,
      so PSUM col 0 accumulates the exact degree (0/1 products in fp32).
      Output columns split 129 + 128 across two PSUM tiles because the
      DoubleRow moving limit is 512 (2*257 = 514 would exceed it).
  finalize j: rec = 1/deg (DVE), x1 = psum * rec -> bf16, PE-transpose x1
      into x1T (bf16), then stage 2: out[k] = [x1 | x] @ W[k] as bf16
      matmuls with W for k=0,1 fused into one 512-wide moving tensor.
      Results are written back as bf16 (cast to f32 on host).
"""

import sys

sys.path.insert(0, "/opt/trn_rl_repo")

import numpy as np

N = 8192
F = 256
O = 256
NCORES = 8
NB = N // NCORES          # nodes per core (1024)
JT = NB // 128            # node tiles per core (8)
MCHUNKS = N // 128        # contraction chunks of 128 (64)
CP = MCHUNKS // 2         # chunk-pairs of 256 for DoubleRow (32)
FP = F + 1                # ones column + x columns (257)


def _patch_tile_drain():
    """This container's walrus allows only one sync-wait per CTRL instruction;
    split the Tile kernel-tail drain's waits onto single-wait no-fuse NoOps."""
    import concourse.tile as tile
    from concourse import mybir
    from concourse.tile import ScopedClock

    if getattr(tile.TileContext, "_drain_split_patched", False):
        return

    def _drain_and_barrier(self, tick_clock, wait_clock):
        nc = self.nc
        drain_inst = nc.sync.drain()
        wait_clock.add_sem_waits(
            drain_inst.ins, ScopedClock({None: tick_clock.global_clock})
        )
        si = drain_inst.ins.sync_info
        if si is not None and len(si.on_wait) > 1:
            waits = list(si.on_wait)
            drain_inst.ins.sync_info = mybir.SyncInfo(
                on_wait=[waits[0]], on_update=list(si.on_update)
            )
            for w in waits[1:]:
                nop = nc.sync.nop(nofuse=True, hint="split_wait")
                nop.ins.sync_info = mybir.SyncInfo(on_wait=[w], on_update=[])
        nc.all_engine_barrier()
        assert self.sems is not None
        popped = nc._tile_sem_poison_stack.pop()
        assert popped is self._sem_poison
        nc.clear_and_free_semaphores(list(self.sems.allocated().values()))
        nc.all_engine_barrier()

    tile.TileContext._drain_and_barrier = _drain_and_barrier
    tile.TileContext._drain_split_patched = True

    # Same walrus limitation, general case: any instruction that Tile gave
    # >1 sem-wait (e.g. a DMA with both RAW and WAR deps) fails codegen.
    # Split surplus waits onto fresh single-wait NoOps emitted just before
    # the instruction on the same engine, at the serialized-BIR level.
    import concourse.bass as bass
    import orjson

    _orig_to_json_bytes = bass.Bass.to_json_bytes

    def _to_json_bytes_split(self):
        m = orjson.loads(_orig_to_json_bytes(self))
        ctr = 0
        for fn in m.get("functions", []):
            for bb in fn.get("blocks", []):
                insts = bb.get("instructions", [])
                # Dedupe redundant PE weight loads: legalization emits one
                # Ldweights per Matmult, but consecutive matmuls that share
                # a stationary tensor (the two output-column groups per mask
                # chunk-pair; the three k's per stage-2 f-chunk) only need
                # the first — the PE array keeps weights across matmuls.
                # A duplicate with sync waits/updates becomes a NoOp that
                # preserves them; a bare one is dropped.
                deduped = []
                cur_key = None
                for inst in insts:
                    if inst.get("engine") != "PE":
                        deduped.append(inst)
                        continue
                    op = inst.get("opcode")
                    if op == "Ldweights":
                        key = orjson.dumps([
                            inst.get("ins"), inst.get("perf_mode"),
                            inst.get("is_transpose"),
                            inst.get("tile_position"), inst.get("tile_size"),
                        ])
                        if key == cur_key:
                            si = inst.get("sync_info")
                            if si and (si.get("on_wait") or si.get("on_update")):
                                deduped.append({
                                    "name": inst["name"] + "-LDWNOP",
                                    "opcode": "NoOp",
                                    "engine": "PE",
                                    "ins": [],
                                    "outs": [],
                                    "sync_info": si,
                                })
                            continue
                        cur_key = key
                    elif op != "Matmult":
                        cur_key = None
                    deduped.append(inst)
                insts = deduped
                new = []
                for inst in insts:
                    si = inst.get("sync_info")
                    waits = (si or {}).get("on_wait") or []
                    if len(waits) > 1:
                        for w in waits[:-1]:
                            ctr += 1
                            new.append({
                                "name": f"SWNOP-{ctr}",
                                "opcode": "NoOp",
                                "engine": inst["engine"],
                                "ins": [],
                                "outs": [],
                                "sync_info": {"on_wait": [w], "on_update": []},
                            })
                        si["on_wait"] = [waits[-1]]
                    new.append(inst)
                bb["instructions"] = new
        return orjson.dumps(m)

    bass.Bass.to_json_bytes = _to_json_bytes_split



def build_bass(with_bias: bool):
    import concourse.bass as bass
    import concourse.tile as tile
    from concourse import mybir
    from concourse.masks import make_identity

    _patch_tile_drain()

    fp8 = mybir.dt.float8e4
    bf16 = mybir.dt.bfloat16
    f32 = mybir.dt.float32
    DR = mybir.MatmulPerfMode.DoubleRow

    nc = bass.Bass()
    maskt = nc.dram_tensor("maskt", [JT, 128, CP * 128 * 2], fp8,
                           kind="ExternalInput")
    xp = nc.dram_tensor("xp", [128, CP * FP * 2], fp8, kind="ExternalInput")
    xt = nc.dram_tensor("xt", [F // 128, 128, NB], bf16, kind="ExternalInput")
    # W for k=0,1 fused along the output dim (512-wide moving tensor); k=2 alone
    w01 = nc.dram_tensor("w01", [2 * F // 128, 128, 2 * O], bf16,
                         kind="ExternalInput")
    w2 = nc.dram_tensor("w2", [2 * F // 128, 128, O], bf16, kind="ExternalInput")
    if with_bias:
        biasr = nc.dram_tensor("biasr", [128, O], f32, kind="ExternalInput")
    out = nc.dram_tensor("out", [JT, 128, 3 * O], bf16, kind="ExternalOutput")

    FCH = 2 * F // 128  # 4 f-chunks of 128 in the stage-2 contraction

    with tile.TileContext(nc) as tc:
        with (
            tc.tile_pool(name="const", bufs=1) as const_pool,
            tc.tile_pool(name="mask", bufs=4) as mask_pool,
            tc.tile_pool(name="work", bufs=3) as work_pool,
            tc.tile_pool(name="psumA", bufs=2, space="PSUM") as psumA_pool,
            tc.tile_pool(name="psumB", bufs=2, space="PSUM") as psumB_pool,
            tc.tile_pool(name="psumt", bufs=2, space="PSUM") as psumt_pool,
            tc.tile_pool(name="psum2", bufs=2, space="PSUM") as psum2_pool,
        ):
            # ---- DMA order matters: everything shares the HWDGE FIFO.
            # First the j=0 mask block interleaved with [1|x] pieces so the
            # PE unblocks at stream rate, then stage-2 constants.
            mt0 = mask_pool.tile([128, CP, 2, 128], fp8, tag="mt", name="mt0")
            xp_sb = const_pool.tile([128, CP, FP, 2], fp8)
            MSPL, XSPL = 4, 8
            mw = CP // MSPL
            xw = CP // XSPL
            for xq in range(XSPL):
                nc.gpsimd.dma_start(
                    xp_sb[:, xq * xw:(xq + 1) * xw, :, :],
                    xp[:, xq * xw * FP * 2:(xq + 1) * xw * FP * 2])
            for q in range(MSPL):
                nc.sync.dma_start(mt0[:, q * mw:(q + 1) * mw, :, :],
                                  maskt[0, :, q * mw * 256:(q + 1) * mw * 256])

            def stage1(j, mt):
                psA = psumA_pool.tile([128, 129], f32, tag="psA")
                psB = psumB_pool.tile([128, 128], f32, tag="psB")
                for c in range(CP):
                    lhsT = mt[:, c, :, :]
                    nc.tensor.matmul(
                        psA[:], lhsT,
                        xp_sb[:, c, 0:129, :].rearrange("p f i -> p i f"),
                        start=(c == 0), stop=(c == CP - 1), perf_mode=DR,
                    )
                    nc.tensor.matmul(
                        psB[:], lhsT,
                        xp_sb[:, c, 129:257, :].rearrange("p f i -> p i f"),
                        start=(c == 0), stop=(c == CP - 1), perf_mode=DR,
                    )
                return psA, psB

            ps0 = stage1(0, mt0)

            # stage-2 constants: emitted after stage1(0), used by finalize(0)
            xt_sb = [const_pool.tile([128, NB], bf16, tag=f"xt{h}", name=f"xt{h}")
                     for h in range(2)]
            for h in range(2):
                nc.scalar.dma_start(xt_sb[h][:], xt[h])
            w01_sb = [const_pool.tile([128, 2 * O], bf16, tag=f"w01_{fc}",
                                      name=f"w01_{fc}") for fc in range(FCH)]
            w2_sb = [const_pool.tile([128, O], bf16, tag=f"w2_{fc}",
                                     name=f"w2_{fc}") for fc in range(FCH)]
            for fc in range(FCH):
                nc.scalar.dma_start(w01_sb[fc][:], w01[fc])
            for fc in range(FCH):
                nc.scalar.dma_start(w2_sb[fc][:], w2[fc])
            if with_bias:
                bias_sb = const_pool.tile([128, O], f32)
                nc.scalar.dma_start(bias_sb[:], biasr[:])
            identity = const_pool.tile([128, 128], bf16)
            make_identity(nc, identity)
            # x1T[h] row f (= h*128+f), col n: x1 transposed, filled per j
            x1t_sb = [const_pool.tile([128, NB], bf16, tag=f"x1t{h}",
                                      name=f"x1t{h}") for h in range(2)]

            def load_mask(j):
                mt = mask_pool.tile([128, CP, 2, 128], fp8, tag="mt",
                                    name=f"mt{j}")
                nc.sync.dma_start(mt[:], maskt[j])
                return mt

            def finalize(j, psA, psB):
                jcols = slice(j * 128, (j + 1) * 128)
                rec = work_pool.tile([128, 1], f32, tag="rec")
                nc.vector.reciprocal(rec[:], psA[:, 0:1])
                x1h = [work_pool.tile([128, 128], bf16, tag=f"x1h{h}",
                                      name=f"x1h{h}") for h in range(2)]
                nc.vector.tensor_scalar_mul(x1h[0][:], psA[:, 1:129], rec[:])
                nc.vector.tensor_scalar_mul(x1h[1][:], psB[:], rec[:])
                for h in range(2):
                    pt = psumt_pool.tile([128, 128], bf16, tag="pt")
                    nc.tensor.transpose(pt[:], x1h[h][:], identity[:])
                    nc.vector.tensor_copy(x1t_sb[h][:, jcols], pt[:])
                lhs = [x1t_sb[0], x1t_sb[1], xt_sb[0], xt_sb[1]]
                po01 = psum2_pool.tile([128, 2 * O], f32, tag="po")
                po2 = psum2_pool.tile([128, 2 * O], f32, tag="po")
                for fc in range(FCH):
                    nc.tensor.matmul(
                        po01[:], lhs[fc][:, jcols], w01_sb[fc][:],
                        start=(fc == 0), stop=(fc == FCH - 1),
                    )
                    nc.tensor.matmul(
                        po2[:, 0:O], lhs[fc][:, jcols], w2_sb[fc][:],
                        start=(fc == 0), stop=(fc == FCH - 1),
                    )
                ot = work_pool.tile([128, 3 * O], bf16, tag="ot")
                if with_bias:
                    nc.vector.tensor_add(ot[:, 0:O], po01[:, 0:O], bias_sb[:])
                    nc.vector.tensor_add(ot[:, O:2 * O], po01[:, O:2 * O],
                                         bias_sb[:])
                    nc.vector.tensor_add(ot[:, 2 * O:], po2[:, 0:O], bias_sb[:])
                else:
                    nc.vector.tensor_copy(ot[:, 0:2 * O], po01[:])
                    nc.vector.tensor_copy(ot[:, 2 * O:], po2[:, 0:O])
                nc.gpsimd.dma_start(out[j], ot[:])

            # software-pipeline by one node-tile so PE never stalls on the
            # DVE reciprocal/divide between stage-1 accumulation and stage 2
            prev = (0, *ps0)
            for j in range(1, JT):
                mt = load_mask(j)
                ps = stage1(j, mt)
                finalize(*prev)
                prev = (j, *ps)
            finalize(*prev)

    return nc


_cached = {}


def _get_bass(with_bias: bool):
    if with_bias not in _cached:
        _cached[with_bias] = build_bass(with_bias)
    return _cached[with_bias]


def _host_prep(x, adj, weight, bias):
    import ml_dtypes

    fp8 = ml_dtypes.float8_e4m3
    bf16 = ml_dtypes.bfloat16
    x = np.asarray(x, dtype=np.float32)
    adj = np.asarray(adj)
    weight = np.asarray(weight, dtype=np.float32)
    bias = np.asarray(bias, dtype=np.float32)

    with_bias = bool(np.any(bias))

    # replicated: [1 | x] in stage-1 layout [128 p][chunk c][1+F], fp8
    xpf = np.empty((N, FP), dtype=np.float32)
    xpf[:, 0] = 1.0
    xpf[:, 1:] = x
    # [p][cp][f][i]: pair elements (i = which 128-block of the 256-chunk)
    # adjacent in SBUF so the PE's DoubleRow moving fetch reads 2 B/column
    xp_t = np.ascontiguousarray(
        xpf.reshape(CP, 2, 128, FP).transpose(2, 0, 3, 1)
    ).reshape(128, CP * FP * 2).astype(fp8)

    # stage-2 weights: k=0,1 fused along output dim; k=2 separate (bf16)
    w_r = weight.reshape(3, 2 * F // 128, 128, O)
    w01_t = np.ascontiguousarray(
        w_r[0:2].transpose(1, 2, 0, 3)
    ).reshape(2 * F // 128, 128, 2 * O).astype(bf16)
    w2_t = np.ascontiguousarray(w_r[2]).astype(bf16)
    bias_r = np.broadcast_to(bias, (128, O)).copy() if with_bias else None

    mask = (adj == 1)
    in_maps = []
    for c in range(NCORES):
        rows = slice(c * NB, (c + 1) * NB)
        # adjT shard in layout [j][p][c][n]: element
        # [j, p, c*128 + n] = mask[node j*128+n, m=c*128+p], fp8
        a = mask[rows].T.astype(fp8)                   # [N m, NB n]
        a = a.reshape(CP, 2, 128, JT, 128)             # [cp, i, p, j, n]
        a = np.ascontiguousarray(a.transpose(3, 2, 0, 1, 4)).reshape(
            JT, 128, CP * 128 * 2
        )
        xt_c = np.ascontiguousarray(x[rows].T).reshape(F // 128, 128, NB
                                                       ).astype(bf16)
        m = {"maskt": a, "xp": xp_t, "xt": xt_c, "w01": w01_t, "w2": w2_t}
        if with_bias:
            m["biasr"] = bias_r
        in_maps.append(m)
    return in_maps, with_bias


def run(x, adj, weight, bias, trace=False, trace_kwargs=None):
    """Shard, run on 8 cores, gather. Returns (out_full, BassKernelResults)."""
    from concourse.bass_utils import run_bass_kernel_spmd

    in_maps, with_bias = _host_prep(x, adj, weight, bias)
    nc = _get_bass(with_bias)
    res = run_bass_kernel_spmd(
        nc, in_maps, list(range(NCORES)), trace=trace, **(trace_kwargs or {})
    )
    out_full = np.empty((3, N, O), dtype=np.float32)
    for c in range(NCORES):
        o = np.asarray(res.results[c]["out"], dtype=np.float32)
        out_full[:, c * NB:(c + 1) * NB, :] = o.reshape(
            NB, 3, O).transpose(1, 0, 2)
    return out_full, res


def kernel(g, x, adj, weight, bias):
    out, _ = run(x, adj, weight, bias)
    return out


# revision 9
# speedup vs baseline: 1.0595x; 1.0595x over previous
"""Trainium2 Bass kernel for CoreSageLayer (GNN mean-aggregate + 3-way linear).

Computation (reference):
    mask = (adj == 1)                      # [N, N] 0/1
    deg  = mask.sum(axis=1)                # [N]
    x1   = (mask @ x) / deg[:, None]       # [N, F]
    out[k] = concat([x1, x], 1) @ W[k] + bias   # [3, N, O]

Distribution: row-shard adj / x1 / out over nodes across 8 cores; replicate
x and weights; no collectives (rows independent).

Device schedule per core (nodes NB=1024, 8 node-tiles of 128):
  stage 1 (per node-tile j): fp8e4m3 DoubleRow matmuls — each instruction
      contracts a 256-neighbor chunk-pair (2 fp8 rows per PE cell, 0.5
      cycles per output column). lhsT = maskT chunk [128, 2, 128], rhs =
      [1 | x] fp8 chunk. Column 0 of the moving tensor is the ones column,
      so PSUM col 0 accumulates the exact degree (0/1 products in fp32).
      Output columns split 129 + 128 across two PSUM tiles because the
      DoubleRow moving limit is 512 (2*257 = 514 would exceed it).
  finalize j: rec = 1/deg (DVE), x1 = psum * rec -> bf16, PE-transpose x1
      into x1T (bf16), then stage 2: out[k] = [x1 | x] @ W[k] as bf16
      matmuls with W for k=0,1 fused into one 512-wide moving tensor.
      Results are written back as bf16 (cast to f32 on host).
"""

import sys

sys.path.insert(0, "/opt/trn_rl_repo")

import numpy as np

N = 8192
F = 256
O = 256
NCORES = 8
NB = N // NCORES          # nodes per core (1024)
JT = NB // 128            # node tiles per core (8)
MCHUNKS = N // 128        # contraction chunks of 128 (64)
CP = MCHUNKS // 2         # chunk-pairs of 256 for DoubleRow (32)
FP = F + 1                # ones column + x columns (257)


def _patch_tile_drain():
    """This container's walrus allows only one sync-wait per CTRL instruction;
    split the Tile kernel-tail drain's waits onto single-wait no-fuse NoOps."""
    import concourse.tile as tile
    from concourse import mybir
    from concourse.tile import ScopedClock

    if getattr(tile.TileContext, "_drain_split_patched", False):
        return

    def _drain_and_barrier(self, tick_clock, wait_clock):
        nc = self.nc
        drain_inst = nc.sync.drain()
        wait_clock.add_sem_waits(
            drain_inst.ins, ScopedClock({None: tick_clock.global_clock})
        )
        si = drain_inst.ins.sync_info
        if si is not None and len(si.on_wait) > 1:
            waits = list(si.on_wait)
            drain_inst.ins.sync_info = mybir.SyncInfo(
                on_wait=[waits[0]], on_update=list(si.on_update)
            )
            for w in waits[1:]:
                nop = nc.sync.nop(nofuse=True, hint="split_wait")
                nop.ins.sync_info = mybir.SyncInfo(on_wait=[w], on_update=[])
        nc.all_engine_barrier()
        assert self.sems is not None
        popped = nc._tile_sem_poison_stack.pop()
        assert popped is self._sem_poison
        nc.clear_and_free_semaphores(list(self.sems.allocated().values()))
        nc.all_engine_barrier()

    tile.TileContext._drain_and_barrier = _drain_and_barrier
    tile.TileContext._drain_split_patched = True

    # Same walrus limitation, general case: any instruction that Tile gave
    # >1 sem-wait (e.g. a DMA with both RAW and WAR deps) fails codegen.
    # Split surplus waits onto fresh single-wait NoOps emitted just before
    # the instruction on the same engine, at the serialized-BIR level.
    import concourse.bass as bass
    import orjson

    _orig_to_json_bytes = bass.Bass.to_json_bytes

    def _to_json_bytes_split(self):
        m = orjson.loads(_orig_to_json_bytes(self))
        ctr = 0
        for fn in m.get("functions", []):
            for bb in fn.get("blocks", []):
                insts = bb.get("instructions", [])
                # Dedupe redundant PE weight loads: legalization emits one
                # Ldweights per Matmult, but consecutive matmuls that share
                # a stationary tensor (the two output-column groups per mask
                # chunk-pair; the three k's per stage-2 f-chunk) only need
                # the first — the PE array keeps weights across matmuls.
                # A duplicate with sync waits/updates becomes a NoOp that
                # preserves them; a bare one is dropped.
                deduped = []
                cur_key = None
                for inst in insts:
                    if inst.get("engine") != "PE":
                        deduped.append(inst)
                        continue
                    op = inst.get("opcode")
                    if op == "Ldweights":
                        key = orjson.dumps([
                            inst.get("ins"), inst.get("perf_mode"),
                            inst.get("is_transpose"),
                            inst.get("tile_position"), inst.get("tile_size"),
                        ])
                        if key == cur_key:
                            si = inst.get("sync_info")
                            if si and (si.get("on_wait") or si.get("on_update")):
                                deduped.append({
                                    "name": inst["name"] + "-LDWNOP",
                                    "opcode": "NoOp",
                                    "engine": "PE",
                                    "ins": [],
                                    "outs": [],
                                    "sync_info": si,
                                })
                            continue
                        cur_key = key
                    elif op != "Matmult":
                        cur_key = None
                    deduped.append(inst)
                insts = deduped
                new = []
                for inst in insts:
                    si = inst.get("sync_info")
                    waits = (si or {}).get("on_wait") or []
                    if len(waits) > 1:
                        for w in waits[:-1]:
                            ctr += 1
                            new.append({
                                "name": f"SWNOP-{ctr}",
                                "opcode": "NoOp",
                                "engine": inst["engine"],
                                "ins": [],
                                "outs": [],
                                "sync_info": {"on_wait": [w], "on_update": []},
                            })
                        si["on_wait"] = [waits[-1]]
                    new.append(inst)
                bb["instructions"] = new
        return orjson.dumps(m)

    bass.Bass.to_json_bytes = _to_json_bytes_split



def build_bass(with_bias: bool):
    import concourse.bass as bass
    import concourse.tile as tile
    from concourse import mybir
    from concourse.masks import make_identity

    _patch_tile_drain()

    fp8 = mybir.dt.float8e4
    bf16 = mybir.dt.bfloat16
    f32 = mybir.dt.float32
    DR = mybir.MatmulPerfMode.DoubleRow

    nc = bass.Bass()
    maskt = nc.dram_tensor("maskt", [JT, 128, CP * 128 * 2], fp8,
                           kind="ExternalInput")
    xp = nc.dram_tensor("xp", [128, CP * FP * 2], fp8, kind="ExternalInput")
    xt = nc.dram_tensor("xt", [F // 128, 128, NB], bf16, kind="ExternalInput")
    # x1-part weights (f < 256) as fp8 pairs for the DoubleRow contraction;
    # x-part weights (f >= 256) in bf16, k=0,1 fused along the output dim
    w1f8 = nc.dram_tensor("w1f8", [128, 3 * O * 2], fp8, kind="ExternalInput")
    w01x = nc.dram_tensor("w01x", [2, 128, 2 * O], bf16, kind="ExternalInput")
    w2x = nc.dram_tensor("w2x", [2, 128, O], bf16, kind="ExternalInput")
    if with_bias:
        biasr = nc.dram_tensor("biasr", [128, O], f32, kind="ExternalInput")
    out = nc.dram_tensor("out", [JT, 128, 3 * O], bf16, kind="ExternalOutput")

    FCH = 2 * F // 128  # 4 f-chunks of 128 in the stage-2 contraction

    with tile.TileContext(nc) as tc:
        with (
            tc.tile_pool(name="const", bufs=1) as const_pool,
            tc.tile_pool(name="mask", bufs=6) as mask_pool,
            tc.tile_pool(name="work", bufs=3) as work_pool,
            tc.tile_pool(name="psumA", bufs=2, space="PSUM") as psumA_pool,
            tc.tile_pool(name="psumB", bufs=2, space="PSUM") as psumB_pool,
            tc.tile_pool(name="psumt", bufs=2, space="PSUM") as psumt_pool,
            tc.tile_pool(name="psum2", bufs=2, space="PSUM") as psum2_pool,
        ):
            # ---- DMA plan: three HWDGE queues.
            #   sync   : mask j0 (pieces), j1, j2, j4, j6
            #   scalar : stage-2 constants, then mask j3, j5, j7
            #   gpsimd : [1|x] pieces, then per-tile output writes
            # First pieces are small so the PE unblocks right after the
            # queues come up.
            mt0 = mask_pool.tile([128, CP, 2, 128], fp8, tag="mt", name="mt0")
            xp_sb = const_pool.tile([128, CP, FP, 2], fp8)
            PIECES = [2, 2, 4, 8, 16]
            off = 0
            for w in PIECES:
                nc.gpsimd.dma_start(
                    xp_sb[:, off:off + w, :, :],
                    xp[:, off * FP * 2:(off + w) * FP * 2])
                off += w
            off = 0
            for w in PIECES:
                nc.sync.dma_start(mt0[:, off:off + w, :, :],
                                  maskt[0, :, off * 256:(off + w) * 256])
                off += w

            def stage1(j, mt, mids):
                psA = psumA_pool.tile([128, 129], f32, tag="psA")
                psB = psumB_pool.tile([128, 128], f32, tag="psB")
                for c in range(CP):
                    if c in mids:
                        mids[c]()
                    lhsT = mt[:, c, :, :]
                    nc.tensor.matmul(
                        psA[:], lhsT,
                        xp_sb[:, c, 0:129, :].rearrange("p f i -> p i f"),
                        start=(c == 0), stop=(c == CP - 1), perf_mode=DR,
                    )
                    nc.tensor.matmul(
                        psB[:], lhsT,
                        xp_sb[:, c, 129:257, :].rearrange("p f i -> p i f"),
                        start=(c == 0), stop=(c == CP - 1), perf_mode=DR,
                    )
                return psA, psB

            ps0 = stage1(0, mt0, {})

            # stage-2 constants (scalar queue, ahead of the odd mask tiles):
            # x1-part weights in fp8 (pairs interleaved along the moving dim),
            # x-part weights + xT in bf16
            w1f8_sb = const_pool.tile([128, 3 * O, 2], fp8)
            nc.scalar.dma_start(w1f8_sb[:], w1f8[:])
            xt_sb = [const_pool.tile([128, NB], bf16, tag=f"xt{h}", name=f"xt{h}")
                     for h in range(2)]
            for h in range(2):
                nc.scalar.dma_start(xt_sb[h][:], xt[h])
            w01x_sb = [const_pool.tile([128, 2 * O], bf16, tag=f"w01x_{fc}",
                                       name=f"w01x_{fc}") for fc in range(2)]
            w2x_sb = [const_pool.tile([128, O], bf16, tag=f"w2x_{fc}",
                                      name=f"w2x_{fc}") for fc in range(2)]
            for fc in range(2):
                nc.scalar.dma_start(w01x_sb[fc][:], w01x[fc])
            for fc in range(2):
                nc.scalar.dma_start(w2x_sb[fc][:], w2x[fc])
            if with_bias:
                bias_sb = const_pool.tile([128, O], f32)
                nc.scalar.dma_start(bias_sb[:], biasr[:])
            identity = const_pool.tile([128, 128], bf16)
            make_identity(nc, identity)
            # x1T planes: x1t_sb[p, i, n] = x1[n, i*128 + p], fp8 for the
            # DoubleRow stage-2 contraction (stationary planes separated)
            x1t_sb = const_pool.tile([128, 2, NB], fp8)

            MQ = {1: nc.sync, 2: nc.sync, 3: nc.scalar, 4: nc.sync,
                  5: nc.scalar, 6: nc.sync, 7: nc.scalar}

            def load_mask(j):
                mt = mask_pool.tile([128, CP, 2, 128], fp8, tag="mt",
                                    name=f"mt{j}")
                MQ[j].dma_start(mt[:], maskt[j])
                return mt

            def fin_a(j, psA, psB):
                # 1/deg scale + transpose x1 into fp8 planes
                jcols = slice(j * 128, (j + 1) * 128)
                rec = work_pool.tile([128, 1], f32, tag="rec")
                nc.vector.reciprocal(rec[:], psA[:, 0:1])
                x1h = [work_pool.tile([128, 128], bf16, tag=f"x1h{h}",
                                      name=f"x1h{h}") for h in range(2)]
                nc.vector.tensor_scalar_mul(x1h[0][:], psA[:, 1:129], rec[:])
                nc.vector.tensor_scalar_mul(x1h[1][:], psB[:], rec[:])
                for h in range(2):
                    pt = psumt_pool.tile([128, 128], bf16, tag="pt")
                    nc.tensor.transpose(pt[:], x1h[h][:], identity[:])
                    nc.vector.tensor_copy(x1t_sb[:, h, jcols], pt[:])

            def fin_b(j):
                # stage 2: out[k] = x1 @ W1[k] (fp8 DoubleRow, one weight
                # load for all three k) + x @ W2[k] (bf16) + bias
                jcols = slice(j * 128, (j + 1) * 128)
                po01 = psum2_pool.tile([128, 2 * O], f32, tag="po")
                po2 = psum2_pool.tile([128, 2 * O], f32, tag="po")
                lhsDR = x1t_sb[:, :, jcols]
                targets = [(po01, 0), (po01, O), (po2, 0)]
                for k, (po, o0) in enumerate(targets):
                    nc.tensor.matmul(
                        po[:, o0:o0 + O], lhsDR,
                        w1f8_sb[:, k * O:(k + 1) * O, :].rearrange(
                            "p o i -> p i o"),
                        start=True, stop=False, perf_mode=DR,
                        skip_group_check=True,
                    )
                for fc in range(2):
                    nc.tensor.matmul(
                        po01[:], xt_sb[fc][:, jcols], w01x_sb[fc][:],
                        start=False, stop=(fc == 1), skip_group_check=True,
                    )
                    nc.tensor.matmul(
                        po2[:, 0:O], xt_sb[fc][:, jcols], w2x_sb[fc][:],
                        start=False, stop=(fc == 1), skip_group_check=True,
                    )
                ot = work_pool.tile([128, 3 * O], bf16, tag="ot")
                if with_bias:
                    nc.vector.tensor_add(ot[:, 0:O], po01[:, 0:O], bias_sb[:])
                    nc.vector.tensor_add(ot[:, O:2 * O], po01[:, O:2 * O],
                                         bias_sb[:])
                    nc.vector.tensor_add(ot[:, 2 * O:], po2[:, 0:O], bias_sb[:])
                else:
                    nc.vector.tensor_copy(ot[:, 0:2 * O], po01[:])
                    nc.vector.tensor_copy(ot[:, 2 * O:], po2[:, 0:O])
                nc.gpsimd.dma_start(out[j], ot[:])

            # software-pipeline by one node-tile: the previous tile's
            # finalize work is injected into this tile's stage-1 chunk loop
            # (transposes at chunk 2, stage-2 at chunk 8) so the PE never
            # stalls on the DVE chain and the kernel tail stays short.
            import functools
            prev = (0, *ps0)
            for j in range(1, JT):
                mt = load_mask(j)
                pj, pA, pB = prev
                mids = {
                    2: functools.partial(fin_a, pj, pA, pB),
                    8: functools.partial(fin_b, pj),
                }
                ps = stage1(j, mt, mids)
                prev = (j, *ps)
            fin_a(*prev)
            fin_b(prev[0])

    return nc


_cached = {}


def _get_bass(with_bias: bool):
    if with_bias not in _cached:
        _cached[with_bias] = build_bass(with_bias)
    return _cached[with_bias]


def _host_prep(x, adj, weight, bias):
    import ml_dtypes

    fp8 = ml_dtypes.float8_e4m3
    bf16 = ml_dtypes.bfloat16
    x = np.asarray(x, dtype=np.float32)
    adj = np.asarray(adj)
    weight = np.asarray(weight, dtype=np.float32)
    bias = np.asarray(bias, dtype=np.float32)

    with_bias = bool(np.any(bias))

    # replicated: [1 | x] in stage-1 layout [128 p][chunk c][1+F], fp8
    xpf = np.empty((N, FP), dtype=np.float32)
    xpf[:, 0] = 1.0
    xpf[:, 1:] = x
    # [p][cp][f][i]: pair elements (i = which 128-block of the 256-chunk)
    # adjacent in SBUF so the PE's DoubleRow moving fetch reads 2 B/column
    xp_t = np.ascontiguousarray(
        xpf.reshape(CP, 2, 128, FP).transpose(2, 0, 3, 1)
    ).reshape(128, CP * FP * 2).astype(fp8)

    # stage-2 weights: x1-part (f < 256) fp8 with pairs interleaved,
    # w1f8[p, k*O+o, i] = W[k, i*128+p, o]; x-part bf16, k=0,1 fused
    w1f8_t = np.ascontiguousarray(
        weight[:, 0:F, :].reshape(3, 2, 128, O).transpose(2, 0, 3, 1)
    ).reshape(128, 3 * O * 2).astype(fp8)
    w_rx = weight[:, F:2 * F, :].reshape(3, 2, 128, O)
    w01x_t = np.ascontiguousarray(
        w_rx[0:2].transpose(1, 2, 0, 3)
    ).reshape(2, 128, 2 * O).astype(bf16)
    w2x_t = np.ascontiguousarray(w_rx[2]).astype(bf16)
    bias_r = np.broadcast_to(bias, (128, O)).copy() if with_bias else None

    mask = (adj == 1)
    in_maps = []
    for c in range(NCORES):
        rows = slice(c * NB, (c + 1) * NB)
        # adjT shard in layout [j][p][c][n]: element
        # [j, p, c*128 + n] = mask[node j*128+n, m=c*128+p], fp8
        a = mask[rows].T.astype(fp8)                   # [N m, NB n]
        a = a.reshape(CP, 2, 128, JT, 128)             # [cp, i, p, j, n]
        a = np.ascontiguousarray(a.transpose(3, 2, 0, 1, 4)).reshape(
            JT, 128, CP * 128 * 2
        )
        xt_c = np.ascontiguousarray(x[rows].T).reshape(F // 128, 128, NB
                                                       ).astype(bf16)
        m = {"maskt": a, "xp": xp_t, "xt": xt_c, "w1f8": w1f8_t,
             "w01x": w01x_t, "w2x": w2x_t}
        if with_bias:
            m["biasr"] = bias_r
        in_maps.append(m)
    return in_maps, with_bias


def run(x, adj, weight, bias, trace=False, trace_kwargs=None):
    """Shard, run on 8 cores, gather. Returns (out_full, BassKernelResults)."""
    from concourse.bass_utils import run_bass_kernel_spmd

    in_maps, with_bias = _host_prep(x, adj, weight, bias)
    nc = _get_bass(with_bias)
    res = run_bass_kernel_spmd(
        nc, in_maps, list(range(NCORES)), trace=trace, **(trace_kwargs or {})
    )
    out_full = np.empty((3, N, O), dtype=np.float32)
    for c in range(NCORES):
        o = np.asarray(res.results[c]["out"], dtype=np.float32)
        out_full[:, c * NB:(c + 1) * NB, :] = o.reshape(
            NB, 3, O).transpose(1, 0, 2)
    return out_full, res


def kernel(g, x, adj, weight, bias):
    out, _ = run(x, adj, weight, bias)
    return out


# revision 10
# speedup vs baseline: 1.0616x; 1.0020x over previous
"""Trainium2 Bass kernel for CoreSageLayer (GNN mean-aggregate + 3-way linear).

Computation (reference):
    mask = (adj == 1)                      # [N, N] 0/1
    deg  = mask.sum(axis=1)                # [N]
    x1   = (mask @ x) / deg[:, None]       # [N, F]
    out[k] = concat([x1, x], 1) @ W[k] + bias   # [3, N, O]

Distribution: row-shard adj / x1 / out over nodes across 8 cores; replicate
x and weights; no collectives (rows independent).

Device schedule per core (nodes NB=1024, 8 node-tiles of 128):
  stage 1 (per node-tile j): fp8e4m3 DoubleRow matmuls — each instruction
      contracts a 256-neighbor chunk-pair (2 fp8 rows per PE cell, 0.5
      cycles per output column). lhsT = maskT chunk [128, 2, 128], rhs =
      [1 | x] fp8 chunk. Column 0 of the moving tensor is the ones column,
      so PSUM col 0 accumulates the exact degree (0/1 products in fp32).
      Output columns split 129 + 128 across two PSUM tiles because the
      DoubleRow moving limit is 512 (2*257 = 514 would exceed it).
  finalize j: rec = 1/deg (DVE), x1 = psum * rec -> bf16, PE-transpose x1
      into x1T (bf16), then stage 2: out[k] = [x1 | x] @ W[k] as bf16
      matmuls with W for k=0,1 fused into one 512-wide moving tensor.
      Results are written back as bf16 (cast to f32 on host).
"""

import sys

sys.path.insert(0, "/opt/trn_rl_repo")

import numpy as np

N = 8192
F = 256
O = 256
NCORES = 8
NB = N // NCORES          # nodes per core (1024)
JT = NB // 128            # node tiles per core (8)
MCHUNKS = N // 128        # contraction chunks of 128 (64)
CP = MCHUNKS // 2         # chunk-pairs of 256 for DoubleRow (32)
FP = F + 1                # ones column + x columns (257)


def _patch_tile_drain():
    """This container's walrus allows only one sync-wait per CTRL instruction;
    split the Tile kernel-tail drain's waits onto single-wait no-fuse NoOps."""
    import concourse.tile as tile
    from concourse import mybir
    from concourse.tile import ScopedClock

    if getattr(tile.TileContext, "_drain_split_patched", False):
        return

    def _drain_and_barrier(self, tick_clock, wait_clock):
        nc = self.nc
        drain_inst = nc.sync.drain()
        wait_clock.add_sem_waits(
            drain_inst.ins, ScopedClock({None: tick_clock.global_clock})
        )
        si = drain_inst.ins.sync_info
        if si is not None and len(si.on_wait) > 1:
            waits = list(si.on_wait)
            drain_inst.ins.sync_info = mybir.SyncInfo(
                on_wait=[waits[0]], on_update=list(si.on_update)
            )
            for w in waits[1:]:
                nop = nc.sync.nop(nofuse=True, hint="split_wait")
                nop.ins.sync_info = mybir.SyncInfo(on_wait=[w], on_update=[])
        nc.all_engine_barrier()
        assert self.sems is not None
        popped = nc._tile_sem_poison_stack.pop()
        assert popped is self._sem_poison
        nc.clear_and_free_semaphores(list(self.sems.allocated().values()))
        nc.all_engine_barrier()

    tile.TileContext._drain_and_barrier = _drain_and_barrier
    tile.TileContext._drain_split_patched = True

    # Same walrus limitation, general case: any instruction that Tile gave
    # >1 sem-wait (e.g. a DMA with both RAW and WAR deps) fails codegen.
    # Split surplus waits onto fresh single-wait NoOps emitted just before
    # the instruction on the same engine, at the serialized-BIR level.
    import concourse.bass as bass
    import orjson

    _orig_to_json_bytes = bass.Bass.to_json_bytes

    def _to_json_bytes_split(self):
        m = orjson.loads(_orig_to_json_bytes(self))
        ctr = 0
        for fn in m.get("functions", []):
            for bb in fn.get("blocks", []):
                insts = bb.get("instructions", [])
                # Dedupe redundant PE weight loads: legalization emits one
                # Ldweights per Matmult, but consecutive matmuls that share
                # a stationary tensor (the two output-column groups per mask
                # chunk-pair; the three k's per stage-2 f-chunk) only need
                # the first — the PE array keeps weights across matmuls.
                # A duplicate with sync waits/updates becomes a NoOp that
                # preserves them; a bare one is dropped.
                deduped = []
                cur_key = None
                for inst in insts:
                    if inst.get("engine") != "PE":
                        deduped.append(inst)
                        continue
                    op = inst.get("opcode")
                    if op == "Ldweights":
                        key = orjson.dumps([
                            inst.get("ins"), inst.get("perf_mode"),
                            inst.get("is_transpose"),
                            inst.get("tile_position"), inst.get("tile_size"),
                        ])
                        if key == cur_key:
                            si = inst.get("sync_info")
                            if si and (si.get("on_wait") or si.get("on_update")):
                                deduped.append({
                                    "name": inst["name"] + "-LDWNOP",
                                    "opcode": "NoOp",
                                    "engine": "PE",
                                    "ins": [],
                                    "outs": [],
                                    "sync_info": si,
                                })
                            continue
                        cur_key = key
                    elif op != "Matmult":
                        cur_key = None
                    deduped.append(inst)
                insts = deduped
                new = []
                for inst in insts:
                    si = inst.get("sync_info")
                    waits = (si or {}).get("on_wait") or []
                    if len(waits) > 1:
                        for w in waits[:-1]:
                            ctr += 1
                            new.append({
                                "name": f"SWNOP-{ctr}",
                                "opcode": "NoOp",
                                "engine": inst["engine"],
                                "ins": [],
                                "outs": [],
                                "sync_info": {"on_wait": [w], "on_update": []},
                            })
                        si["on_wait"] = [waits[-1]]
                    new.append(inst)
                bb["instructions"] = new
        return orjson.dumps(m)

    bass.Bass.to_json_bytes = _to_json_bytes_split



def build_bass(with_bias: bool):
    import concourse.bass as bass
    import concourse.tile as tile
    from concourse import mybir
    from concourse.masks import make_identity

    _patch_tile_drain()

    fp8 = mybir.dt.float8e4
    bf16 = mybir.dt.bfloat16
    f32 = mybir.dt.float32
    DR = mybir.MatmulPerfMode.DoubleRow

    nc = bass.Bass()
    maskt = nc.dram_tensor("maskt", [JT, 128, CP * 128 * 2], fp8,
                           kind="ExternalInput")
    xp = nc.dram_tensor("xp", [128, CP * FP * 2], fp8, kind="ExternalInput")
    xt = nc.dram_tensor("xt", [F // 128, 128, NB], bf16, kind="ExternalInput")
    # x1-part weights (f < 256) as fp8 pairs for the DoubleRow contraction;
    # x-part weights (f >= 256) in bf16, k=0,1 fused along the output dim
    w1f8 = nc.dram_tensor("w1f8", [128, 3 * O * 2], fp8, kind="ExternalInput")
    w01x = nc.dram_tensor("w01x", [2, 128, 2 * O], bf16, kind="ExternalInput")
    w2x = nc.dram_tensor("w2x", [2, 128, O], bf16, kind="ExternalInput")
    if with_bias:
        biasr = nc.dram_tensor("biasr", [128, O], f32, kind="ExternalInput")
    out = nc.dram_tensor("out", [JT, 128, 3 * O], bf16, kind="ExternalOutput")

    FCH = 2 * F // 128  # 4 f-chunks of 128 in the stage-2 contraction

    with tile.TileContext(nc) as tc:
        with (
            tc.tile_pool(name="const", bufs=1) as const_pool,
            tc.tile_pool(name="mask", bufs=6) as mask_pool,
            tc.tile_pool(name="work", bufs=3) as work_pool,
            tc.tile_pool(name="psumA", bufs=2, space="PSUM") as psumA_pool,
            tc.tile_pool(name="psumB", bufs=2, space="PSUM") as psumB_pool,
            tc.tile_pool(name="psumt", bufs=2, space="PSUM") as psumt_pool,
            tc.tile_pool(name="psum2", bufs=2, space="PSUM") as psum2_pool,
        ):
            # ---- DMA plan: three HWDGE queues.
            #   sync   : mask j0 (pieces), j1, j2, j4, j6
            #   scalar : stage-2 constants, then mask j3, j5, j7
            #   gpsimd : [1|x] pieces, then per-tile output writes
            # First pieces are small so the PE unblocks right after the
            # queues come up.
            mt0 = mask_pool.tile([128, CP, 2, 128], fp8, tag="mt", name="mt0")
            xp_sb = const_pool.tile([128, CP, FP, 2], fp8)
            PIECES = [2, 2, 4, 8, 16]
            off = 0
            for w in PIECES:
                nc.gpsimd.dma_start(
                    xp_sb[:, off:off + w, :, :],
                    xp[:, off * FP * 2:(off + w) * FP * 2])
                off += w
            off = 0
            for w in PIECES:
                nc.sync.dma_start(mt0[:, off:off + w, :, :],
                                  maskt[0, :, off * 256:(off + w) * 256])
                off += w

            def stage1(j, mt, mids):
                psA = psumA_pool.tile([128, 129], f32, tag="psA")
                psB = psumB_pool.tile([128, 128], f32, tag="psB")
                for c in range(CP):
                    if c in mids:
                        mids[c]()
                    lhsT = mt[:, c, :, :]
                    nc.tensor.matmul(
                        psA[:], lhsT,
                        xp_sb[:, c, 0:129, :].rearrange("p f i -> p i f"),
                        start=(c == 0), stop=(c == CP - 1), perf_mode=DR,
                    )
                    nc.tensor.matmul(
                        psB[:], lhsT,
                        xp_sb[:, c, 129:257, :].rearrange("p f i -> p i f"),
                        start=(c == 0), stop=(c == CP - 1), perf_mode=DR,
                    )
                return psA, psB

            ps0 = stage1(0, mt0, {})

            # stage-2 constants (scalar queue, ahead of the odd mask tiles):
            # x1-part weights in fp8 (pairs interleaved along the moving dim),
            # x-part weights + xT in bf16
            w1f8_sb = const_pool.tile([128, 3 * O, 2], fp8)
            nc.scalar.dma_start(w1f8_sb[:], w1f8[:])
            xt_sb = [const_pool.tile([128, NB], bf16, tag=f"xt{h}", name=f"xt{h}")
                     for h in range(2)]
            for h in range(2):
                nc.scalar.dma_start(xt_sb[h][:], xt[h])
            w01x_sb = [const_pool.tile([128, 2 * O], bf16, tag=f"w01x_{fc}",
                                       name=f"w01x_{fc}") for fc in range(2)]
            w2x_sb = [const_pool.tile([128, O], bf16, tag=f"w2x_{fc}",
                                      name=f"w2x_{fc}") for fc in range(2)]
            for fc in range(2):
                nc.scalar.dma_start(w01x_sb[fc][:], w01x[fc])
            for fc in range(2):
                nc.scalar.dma_start(w2x_sb[fc][:], w2x[fc])
            if with_bias:
                bias_sb = const_pool.tile([128, O], f32)
                nc.scalar.dma_start(bias_sb[:], biasr[:])
            identity = const_pool.tile([128, 128], bf16)
            make_identity(nc, identity)
            # stage-2 runs on 64*x1 / 64*x (keeps x1 out of fp8's subnormal
            # range); the output copy folds the 1/64 back in
            inv64 = const_pool.tile([128, 1], f32)
            nc.gpsimd.memset(inv64[:], 1.0 / 64.0)
            # x1T planes: x1t_sb[p, i, n] = x1[n, i*128 + p], fp8 for the
            # DoubleRow stage-2 contraction (stationary planes separated)
            x1t_sb = const_pool.tile([128, 2, NB], fp8)

            MQ = {1: nc.sync, 2: nc.sync, 3: nc.scalar, 4: nc.sync,
                  5: nc.scalar, 6: nc.sync, 7: nc.scalar}

            def load_mask(j):
                mt = mask_pool.tile([128, CP, 2, 128], fp8, tag="mt",
                                    name=f"mt{j}")
                MQ[j].dma_start(mt[:], maskt[j])
                return mt

            def fin_a(j, psA, psB):
                # 1/deg scale + transpose x1 into fp8 planes
                jcols = slice(j * 128, (j + 1) * 128)
                rec = work_pool.tile([128, 1], f32, tag="rec")
                nc.vector.reciprocal(rec[:], psA[:, 0:1])
                x1h = [work_pool.tile([128, 128], bf16, tag=f"x1h{h}",
                                      name=f"x1h{h}") for h in range(2)]
                nc.vector.tensor_scalar_mul(x1h[0][:], psA[:, 1:129], rec[:])
                nc.vector.tensor_scalar_mul(x1h[1][:], psB[:], rec[:])
                for h in range(2):
                    pt = psumt_pool.tile([128, 128], bf16, tag="pt")
                    nc.tensor.transpose(pt[:], x1h[h][:], identity[:])
                    nc.vector.tensor_copy(x1t_sb[:, h, jcols], pt[:])

            def fin_b(j):
                # stage 2: out[k] = x1 @ W1[k] (fp8 DoubleRow, one weight
                # load for all three k) + x @ W2[k] (bf16) + bias
                jcols = slice(j * 128, (j + 1) * 128)
                po01 = psum2_pool.tile([128, 2 * O], f32, tag="po")
                po2 = psum2_pool.tile([128, 2 * O], f32, tag="po")
                lhsDR = x1t_sb[:, :, jcols]
                targets = [(po01, 0), (po01, O), (po2, 0)]
                for k, (po, o0) in enumerate(targets):
                    nc.tensor.matmul(
                        po[:, o0:o0 + O], lhsDR,
                        w1f8_sb[:, k * O:(k + 1) * O, :].rearrange(
                            "p o i -> p i o"),
                        start=True, stop=False, perf_mode=DR,
                        skip_group_check=True,
                    )
                for fc in range(2):
                    nc.tensor.matmul(
                        po01[:], xt_sb[fc][:, jcols], w01x_sb[fc][:],
                        start=False, stop=(fc == 1), skip_group_check=True,
                    )
                    nc.tensor.matmul(
                        po2[:, 0:O], xt_sb[fc][:, jcols], w2x_sb[fc][:],
                        start=False, stop=(fc == 1), skip_group_check=True,
                    )
                ot = work_pool.tile([128, 3 * O], bf16, tag="ot")
                if with_bias:
                    sc = work_pool.tile([128, 3 * O], f32, tag="sc")
                    nc.vector.tensor_scalar_mul(sc[:, 0:2 * O], po01[:],
                                                inv64[:])
                    nc.vector.tensor_scalar_mul(sc[:, 2 * O:], po2[:, 0:O],
                                                inv64[:])
                    nc.vector.tensor_add(ot[:, 0:O], sc[:, 0:O], bias_sb[:])
                    nc.vector.tensor_add(ot[:, O:2 * O], sc[:, O:2 * O],
                                         bias_sb[:])
                    nc.vector.tensor_add(ot[:, 2 * O:], sc[:, 2 * O:],
                                         bias_sb[:])
                else:
                    nc.vector.tensor_scalar_mul(ot[:, 0:2 * O], po01[:],
                                                inv64[:])
                    nc.vector.tensor_scalar_mul(ot[:, 2 * O:], po2[:, 0:O],
                                                inv64[:])
                nc.gpsimd.dma_start(out[j], ot[:])

            # software-pipeline by one node-tile: the previous tile's
            # finalize work is injected into this tile's stage-1 chunk loop
            # (transposes at chunk 2, stage-2 at chunk 8) so the PE never
            # stalls on the DVE chain and the kernel tail stays short.
            import functools
            prev = (0, *ps0)
            for j in range(1, JT):
                mt = load_mask(j)
                pj, pA, pB = prev
                mids = {
                    2: functools.partial(fin_a, pj, pA, pB),
                    8: functools.partial(fin_b, pj),
                }
                ps = stage1(j, mt, mids)
                prev = (j, *ps)
            fin_a(*prev)
            fin_b(prev[0])

    return nc


_cached = {}


def _get_bass(with_bias: bool):
    if with_bias not in _cached:
        _cached[with_bias] = build_bass(with_bias)
    return _cached[with_bias]


def _host_prep(x, adj, weight, bias):
    import ml_dtypes

    fp8 = ml_dtypes.float8_e4m3
    bf16 = ml_dtypes.bfloat16
    x = np.asarray(x, dtype=np.float32)
    adj = np.asarray(adj)
    weight = np.asarray(weight, dtype=np.float32)
    bias = np.asarray(bias, dtype=np.float32)

    with_bias = bool(np.any(bias))

    # replicated: [1 | x] in stage-1 layout [128 p][chunk c][1+F], fp8
    xpf = np.empty((N, FP), dtype=np.float32)
    xpf[:, 0] = 1.0 / 64.0
    xpf[:, 1:] = x
    # [p][cp][f][i]: pair elements (i = which 128-block of the 256-chunk)
    # adjacent in SBUF so the PE's DoubleRow moving fetch reads 2 B/column
    xp_t = np.ascontiguousarray(
        xpf.reshape(CP, 2, 128, FP).transpose(2, 0, 3, 1)
    ).reshape(128, CP * FP * 2).astype(fp8)

    # stage-2 weights: x1-part (f < 256) fp8 with pairs interleaved,
    # w1f8[p, k*O+o, i] = W[k, i*128+p, o]; x-part bf16, k=0,1 fused
    w1f8_t = np.ascontiguousarray(
        weight[:, 0:F, :].reshape(3, 2, 128, O).transpose(2, 0, 3, 1)
    ).reshape(128, 3 * O * 2).astype(fp8)
    w_rx = weight[:, F:2 * F, :].reshape(3, 2, 128, O)
    w01x_t = np.ascontiguousarray(
        w_rx[0:2].transpose(1, 2, 0, 3)
    ).reshape(2, 128, 2 * O).astype(bf16)
    w2x_t = np.ascontiguousarray(w_rx[2]).astype(bf16)
    bias_r = np.broadcast_to(bias, (128, O)).copy() if with_bias else None

    mask = (adj == 1)
    in_maps = []
    for c in range(NCORES):
        rows = slice(c * NB, (c + 1) * NB)
        # adjT shard in layout [j][p][c][n]: element
        # [j, p, c*128 + n] = mask[node j*128+n, m=c*128+p], fp8
        a = mask[rows].T.astype(fp8)                   # [N m, NB n]
        a = a.reshape(CP, 2, 128, JT, 128)             # [cp, i, p, j, n]
        a = np.ascontiguousarray(a.transpose(3, 2, 0, 1, 4)).reshape(
            JT, 128, CP * 128 * 2
        )
        xt_c = np.ascontiguousarray(64.0 * x[rows].T).reshape(
            F // 128, 128, NB).astype(bf16)
        m = {"maskt": a, "xp": xp_t, "xt": xt_c, "w1f8": w1f8_t,
             "w01x": w01x_t, "w2x": w2x_t}
        if with_bias:
            m["biasr"] = bias_r
        in_maps.append(m)
    return in_maps, with_bias


def run(x, adj, weight, bias, trace=False, trace_kwargs=None):
    """Shard, run on 8 cores, gather. Returns (out_full, BassKernelResults)."""
    from concourse.bass_utils import run_bass_kernel_spmd

    in_maps, with_bias = _host_prep(x, adj, weight, bias)
    nc = _get_bass(with_bias)
    res = run_bass_kernel_spmd(
        nc, in_maps, list(range(NCORES)), trace=trace, **(trace_kwargs or {})
    )
    out_full = np.empty((3, N, O), dtype=np.float32)
    for c in range(NCORES):
        o = np.asarray(res.results[c]["out"], dtype=np.float32)
        out_full[:, c * NB:(c + 1) * NB, :] = o.reshape(
            NB, 3, O).transpose(1, 0, 2)
    return out_full, res


def kernel(g, x, adj, weight, bias):
    out, _ = run(x, adj, weight, bias)
    return out
